# revision 3
# baseline (speedup 1.0000x reference)
"""LIF kernel v7: v6-style recurrence (1 DVE stt/step, DMA-CCE x-add) + PE-
packed output: spikes leave the chip as 4-bit-packed nibbles (2 MB/core
instead of 8 MB), cutting DMA traffic from 40 MB to 34 MB/core.

Recurrence (state p = pre-reset potential, identical to kernel v5, all on
DVE so the accum->DVE->accum loop has only 2 hops):

    DVE A:     m = (p is_ge 1) subtract p      [= o - p, stt in0=in1=p]
    DVE B:     h = m * -0.5                    [= 0.5*u^post, ts 2x mode]
    DMA-CCE:   p_{t+1} = h + x_{t+1}           [the x load accumulates]

Pack path (pure consumer, off the critical loop):

    ACT sign:  s_t(bf16) = Sign(p_t - 1)       [{-1, 0(measure-zero), +1}]
    PE:        psum[32g:32g+32] = Wp.T @ s     [Wp (128,32) bf16, powers of
                                                2 over channel groups of 4]
    ACT cast:  pv(u8) = Copy(0.5*psum + 7.5)   [= sum_j 2^j o_j, exact
                                                integer nibbles 0..15]

Bit-exact vs the f32 jax reference (same rounding sequence as v5); only
p == 1.0 exactly (sign(0) = 0) can perturb one output nibble.
"""

import numpy as np

B, T, C, HW = 32, 16, 128, 1024
NCORES = 8
BLOC = B // NCORES   # 4
FREE = BLOC * HW     # 4096
NCH = 4              # ACT/DVE chunks (1024 wide)
CH = FREE // NCH
NH = 8               # PE half-chunks (512 wide)
HC = FREE // NH
NO = 2               # s (bf16) slots
NBK = 4              # psum banks in rotation

_cached = {}


def _build_nc():
    import concourse.bass as bass
    import concourse.mybir as mybir
    from contextlib import ExitStack

    f32 = mybir.dt.float32
    bf16 = mybir.dt.bfloat16
    u8 = mybir.dt.uint8
    Alu = mybir.AluOpType
    Act = mybir.ActivationFunctionType

    nc = bass.Bass()
    _bias = nc.alloc_sbuf_tensor("const-f32-neg1", [128, 1], f32)
    nc.const_aps.aps[(f32, -1.0)] = _bias.ap()

    x_d = nc.declare_dram_parameter("x", [BLOC, T, C, HW], f32, isOutput=False)
    b_d = nc.declare_dram_parameter("bias", [C, 1], f32, isOutput=False)
    w_d = nc.declare_dram_parameter("wp", [C, 32], bf16, isOutput=False)
    # packed output: [T, s(2), 128, 512] u8 nibbles
    po_d = nc.declare_dram_parameter("po", [T, 2, C, HC], u8, isOutput=True)

    def dram_chunk(d, t, c):
        k = HW // CH
        return d[c // k, t, :, (c % k) * CH : (c % k + 1) * CH]

    with ExitStack() as ctx:
        p = [
            ctx.enter_context(nc.sbuf_tensor(f"p{i}", [C, FREE], f32))
            for i in range(2)
        ]
        ot = [
            ctx.enter_context(nc.sbuf_tensor(f"ot{i}", [C, FREE], bf16))
            for i in range(NO)
        ]
        m = ctx.enter_context(nc.sbuf_tensor("m", [C, FREE], f32))
        xq = ctx.enter_context(nc.sbuf_tensor("xq", [C, FREE], f32))
        wp = ctx.enter_context(nc.sbuf_tensor("wp_sb", [C, 32], bf16))
        pv = [
            ctx.enter_context(nc.sbuf_tensor(f"pv{i}", [C, HC], u8))
            for i in range(2)
        ]
        pk = [
            ctx.enter_context(nc.psum_tensor(f"pk{i}", [C, HC], f32))
            for i in range(NBK)
        ]
        s_p = [
            [ctx.enter_context(nc.semaphore(f"s_p{i}_{c}")) for c in range(NCH)]
            for i in range(2)
        ]
        s_dve = ctx.enter_context(nc.semaphore("s_dve"))
        s_act = ctx.enter_context(nc.semaphore("s_act"))
        s_pe = ctx.enter_context(nc.semaphore("s_pe"))
        s_cast = ctx.enter_context(nc.semaphore("s_cast"))
        s_op = [ctx.enter_context(nc.semaphore(f"s_op{s}"))
                for s in range(2)]
        s_init = ctx.enter_context(nc.semaphore("s_init"))
        s_q = [ctx.enter_context(nc.semaphore(f"s_q{c}")) for c in range(2)]
        block = ctx.enter_context(nc.Block())

        def chunk(ap, c):
            return ap[:, c * CH : (c + 1) * CH]

        def half(ap, h):
            return ap[:, h * HC : (h + 1) * HC]

        # counts:
        #  s_act: sign ACT_t[c] -> 4t + c + 1
        #  s_dve: A_t[c] -> 8t + 2c + 1, B_t[c] -> 8t + 2c + 2, t in 0..T-2
        #  s_pe:  MM(t, h) -> 8t + h + 1
        #  s_cast: cast(t, s) -> 2t + s + 1
        #  s_outp: out-DMA(t, s) -> 32t + 16(s+1)

        @block.gpsimd
        def _(gpsimd: bass.BassEngine):
            for t in range(1, T):
                for c in range(NCH if t < T - 1 else 2):
                    gpsimd.wait_ge(s_dve, 2 * NCH * (t - 1) + 2 * c + 2)
                    gpsimd.dma_start(
                        out=chunk(p[t % 2], c),
                        in_=dram_chunk(x_d, t, c),
                        accum_op=Alu.add,
                    ).then_inc(s_p[t % 2][c], 16)

        @block.vector
        def _(vector: bass.BassEngine):
            # Recurrence entirely on DVE (2-hop loop with the accum-DMA),
            # state = p (pre-reset potential), exactly as kernel v5:
            #   A_t[c]: m = (p is_ge 1) - p   (= o - p = -u^post)
            #   B_t[c]: p' = m * -0.5         (= 0.5*u^post; x added by CCE)
            for t in range(T - 1):
                for c in range(NCH):
                    tail = t == T - 2 and c >= 2
                    vector.wait_ge(s_p[t % 2][c], 16 * (t // 2 + 1))
                    vector.scalar_tensor_tensor(
                        out=chunk(m, c), in0=chunk(p[t % 2], c), scalar=1.0,
                        in1=chunk(p[t % 2], c),
                        op0=Alu.is_ge, op1=Alu.subtract,
                    ).then_inc(s_dve, 1)
                    if t >= 1:
                        vector.wait_ge(s_act, NCH * (t - 1) + c + 1)
                    # tail chunks at t=14 store p'' = +0.5m; o15 compares
                    # prefetched x15 against it, skipping the last accum+sign
                    vector.tensor_scalar(
                        out=chunk(p[(t + 1) % 2], c), in0=chunk(m, c),
                        scalar1=(0.5 if tail else -0.5), scalar2=None,
                        op0=Alu.mult,
                    ).then_inc(s_dve, 1)
                    if tail:
                        # o15 = ((x15 - 1) >= p'') = (h + x15 >= 1) in {0,1}
                        vector.wait_ge(s_q[c - 2], 16)
                        vector.wait_ge(s_pe, NH * (T - 3) + 2 * c + 2)
                        vector.scalar_tensor_tensor(
                            out=chunk(ot[(T - 1) % NO], c),
                            in0=chunk(xq, c), scalar=1.0,
                            in1=chunk(p[(T - 1) % 2], c),
                            op0=Alu.subtract, op1=Alu.is_ge,
                        ).then_inc(s_dve, 1)

        @block.scalar
        def _(scalar: bass.BassEngine):
            scalar.wait_ge(s_init, 16)
            for t in range(T):
                # sign chunks (t=15 chunks 2,3 come from DVE's o15)
                for c in range(NCH if t < T - 1 else 2):
                    scalar.wait_ge(s_p[t % 2][c], 16 * (t // 2 + 1))
                    if t >= NO:
                        # ot[t%2][c] free once MMs(t-2) read it
                        scalar.wait_ge(s_pe, NH * (t - 2) + 2 * c + 2)
                    scalar.activation(
                        out=chunk(ot[t % NO], c), in_=chunk(p[t % 2], c),
                        func=Act.Sign, bias=-1.0, scale=1.0,
                    ).then_inc(s_act, 1)
                # psum casts (2 per t, after the 4 MMs of each bank)
                for s in range(2):
                    scalar.wait_ge(s_pe, NH * t + 4 * (s + 1))
                    if t >= 1:
                        scalar.wait_ge(s_op[s], 16 * t)
                    bk = (2 * t + s) % NBK
                    # t=15 s1 psum holds sum(2^j * o_j) directly (o in {0,1})
                    cb, cs = ((0.0, 1.0) if (t == T - 1 and s == 1)
                              else (7.5, 0.5))
                    scalar.activation(
                        out=pv[s][:, :], in_=pk[bk][:, :],
                        func=Act.Copy, bias=cb, scale=cs,
                    ).then_inc(s_cast, 1)
                    if t == T - 1:
                        scalar.dma_start(
                            out=po_d[t, s], in_=pv[s][:, :]
                        ).then_inc(s_op[s], 16)

        @block.tensor
        def _(tensor: bass.BassEngine):
            tensor.wait_ge(s_init, 32)
            for t in range(T):
                for h in range(NH):
                    s, g = h // 4, h % 4
                    if t == T - 1 and h >= 4:
                        tensor.wait_ge(
                            s_dve, 8 * (T - 2) + (7 if h < 6 else 10))
                    else:
                        tensor.wait_ge(s_act, NCH * t + (h // 2) + 1)
                    if t >= 2:
                        tensor.wait_ge(s_cast, 2 * (t - 2) + s + 1)
                    bk = (2 * t + s) % NBK
                    tensor.matmul(
                        out=pk[bk][32 * g : 32 * g + 32, :],
                        lhsT=wp[:, :],
                        rhs=half(ot[t % NO], h),
                        start=True, stop=True,
                        tile_position=(0, 32 * g),
                    ).then_inc(s_pe, 1)

        @block.sync
        def _(sync: bass.BassEngine):
            for c in range(NCH):
                sync.dma_start(
                    out=chunk(p[0], c), in_=dram_chunk(x_d, 0, c)
                ).then_inc(s_p[0][c], 16)
            sync.dma_start(out=_bias.ap(), in_=b_d[:, :]).then_inc(s_init, 16)
            sync.dma_start(out=wp[:, :], in_=w_d[:, :]).then_inc(s_init, 16)
            for t in range(T - 1):
                for s in range(2):
                    sync.wait_ge(s_cast, 2 * t + s + 1)
                    sync.dma_start(
                        out=po_d[t, s], in_=pv[s][:, :]
                    ).then_inc(s_op[s], 16)
                if 8 <= t <= 9:
                    c = t - 6
                    sync.dma_start(
                        out=chunk(xq, c), in_=dram_chunk(x_d, T - 1, c)
                    ).then_inc(s_q[c - 2], 16)
            sync.wait_ge(s_op[0], 16 * T)
            sync.wait_ge(s_op[1], 16 * T)

    return nc


def _get_nc():
    if "nc" not in _cached:
        _cached["nc"] = _build_nc()
    return _cached["nc"]


def kernel(x_seq: np.ndarray) -> np.ndarray:
    import ml_dtypes
    from concourse.bass_utils import run_bass_kernel_spmd

    x = np.ascontiguousarray(np.asarray(x_seq, dtype=np.float32)).reshape(
        B, T, C, HW
    )
    nc = _get_nc()
    bias = np.full((C, 1), -1.0, dtype=np.float32)
    wp = np.zeros((C, 32), dtype=ml_dtypes.bfloat16)
    for pp in range(C):
        wp[pp, pp // 4] = float(1 << (pp % 4))
    in_maps = [
        {"x": x[i * BLOC : (i + 1) * BLOC], "bias": bias, "wp": wp}
        for i in range(NCORES)
    ]
    out = run_bass_kernel_spmd(nc, in_maps, list(range(NCORES)))
    _cached["last"] = out
    res = out.results

    o = np.zeros((NCORES, BLOC, T, C, HW), dtype=np.float32)
    for i in range(NCORES):
        v = res[i]["po"]  # [T, 2, 128, 512] u8
        v = v.reshape(T, 2, 4, 32, HC)  # [t, s, g, q, f]
        for s in range(2):
            for g in range(4):
                h = 4 * s + g
                b, hw0 = h // 2, (h % 2) * HC
                blk = v[:, s, g]  # [T, 32, 512]
                bits = (
                    blk[:, :, None, :] >> np.arange(4)[None, None, :, None]
                ) & 1
                # bits: [T, q, j, f] -> channels ch = 4q + j
                o[i, b, :, :, hw0 : hw0 + HC] = bits.reshape(T, C, HC)
    return o.reshape(B, T, C, 32, 32).astype(np.float32)

